# revision 2
# baseline (speedup 1.0000x reference)
"""Trainium2 Bass kernel for nn_Conv3DNorm (modulated conv3d + demod + lrelu + clamp).

Reference math (styles == ones):
    dcoef[cout] = rsqrt(sum_{cin,kd,kh,kw} weight^2 + 1e-8)
    y = conv3d(x, weight * dcoef, pad=1)            # per-sample, stride 1
    y = leaky_relu(y + bias, 0.2) * sqrt(2)
    y = clip(y, -256, 256)

Sharding: data-parallel over batch. Core i processes sample i (B=8 == n_cores).
Weight/bias replicated. Everything on device except input layout/dtype prep:
  - weight pre-transposed on host to [cin, tap, cout] (matmul lhsT layout)
  - conv is computed as 27 accumulated matmuls (one per kernel tap) over a
    zero-padded (H,W)-padded input volume resident in SBUF; depth taps that
    fall outside the volume are skipped (implicit D padding).
  - matmul runs in bf16 by default: measured 216 ns/MM at N=512 (the roofline
    213.3 + 2.5 ns NX issue), vs 244 ns/MM for f32r which pays a ~30 ns
    per-instruction penalty. bf16 in/out keeps rel err ~1e-3 (gate is 2e-2).
  - startup: w DMA split in 3 tap-groups on the sync HWDGE ring; first two
    x depth slices go on the scalar HWDGE ring so chunk 0 can start ~8us in;
    the remaining slices stream on the gpsimd SWDGE queue.
  - epilogue per chunk is 3 DVE ops: u = ps*(sqrt2*dcoef) + sqrt2*bias;
    v = max(0.2*u, u)  (== sqrt2*leaky_relu);  out = clip(v, +-256).
"""

import os
import sys

for _p in (
    "/root/.axon_site",
    "/root/.axon_site/_ro/trn_rl_repo",
    "/root/.axon_site/_ro/pypackages",
):
    if os.path.isdir(_p) and _p not in sys.path:
        sys.path.insert(0, _p)

import numpy as np

import concourse.bass as bass  # noqa: F401
import concourse.mybir as mybir
import concourse.tile as tile
from concourse import bacc
from concourse.bass_utils import run_bass_kernel_spmd

# Problem constants (hardcoded per contract).
B = 8
CIN = 128
COUT = 128
D = H = W = 32
K = 3
NTAPS = K * K * K  # 27
HP = H + 2  # 34
WP = W + 2  # 34
NCHUNK = 64  # output chunks of 512 spatial positions: (d, half-of-H)
EPS = 1e-8
S1 = float(np.sqrt(2.0))  # ACT_GAIN * GAIN
CLAMP = 256.0
ALPHA = 0.2

# matmul dtype: "bf16" (roofline), "f32r" (TF32-like, +30ns/MM), "f32" (4x slower)
MM_MODE = os.environ.get("CONV_MM_MODE", "bf16")

LAST_RESULTS = None  # BassKernelResults of the most recent run (for test.py)

_CACHED = {}


def _build_nc(mode: str):
    dt = mybir.dt
    io_dt = {"f32r": dt.float32r, "bf16": dt.bfloat16, "f32": dt.float32}[mode]

    nc = bacc.Bacc("TRN2")
    x_d = nc.dram_tensor("x", [CIN, D, H, W], io_dt, kind="ExternalInput")
    w_d = nc.dram_tensor("w", [CIN, NTAPS, COUT], io_dt, kind="ExternalInput")
    b_d = nc.dram_tensor("bias", [COUT, 1], dt.float32, kind="ExternalInput")
    y_d = nc.dram_tensor("y", [COUT, NCHUNK, 512], dt.float32, kind="ExternalOutput")

    def asf32(ap):
        return ap.bitcast(dt.float32) if mode == "f32r" else ap

    with tile.TileContext(nc) as tc:
        with (
            tc.tile_pool(name="big", bufs=1) as big,
            tc.tile_pool(name="small", bufs=1) as small,
            tc.tile_pool(name="sq", bufs=2) as sqp,
            tc.tile_pool(name="epiv", bufs=4) as vp,
            tc.tile_pool(name="epio", bufs=4) as op,
        ):
            # ---- padded input volume in SBUF: [cin, d, h+2, w+2] ----
            xpad = big.tile([CIN, D, HP, WP], io_dt)
            # zero the (H,W) halo once (bitcast: memset lacks f32r support).
            # These go first on DVE so they don't gate the first conv matmul.
            nc.vector.memset(asf32(xpad[:, :, 0, :]), 0.0)
            nc.vector.memset(asf32(xpad[:, :, HP - 1, :]), 0.0)
            nc.vector.memset(asf32(xpad[:, :, 1 : HP - 1, 0]), 0.0)
            nc.vector.memset(asf32(xpad[:, :, 1 : HP - 1, WP - 1]), 0.0)

            # ---- weights + bias in SBUF ----
            # w split in 3 tap-groups on the sync HWDGE ring so tap 0 lands
            # ~3us earlier than a monolithic transfer would.
            w_sb = big.tile([CIN, NTAPS, COUT], io_dt)
            for g in range(3):
                nc.sync.dma_start(
                    w_sb[:, 9 * g : 9 * (g + 1), :], w_d[:, 9 * g : 9 * (g + 1), :]
                )
            bias_sb = small.tile([COUT, 1], dt.float32)
            nc.scalar.dma_start(bias_sb[:], b_d[:])

            # ---- input: first two depth slices on the scalar HWDGE ring
            # (chunk 0 needs them); the rest stream on the SWDGE queue.
            for d in range(2):
                nc.scalar.dma_start(
                    xpad[:, d, 1 : HP - 1, 1 : WP - 1], x_d[:, d, :, :]
                )
            for d in range(2, D):
                nc.gpsimd.dma_start(
                    xpad[:, d, 1 : HP - 1, 1 : WP - 1], x_d[:, d, :, :]
                )

            # ---- demodulation coefficients (emitted after chunk 0's matmuls
            # so the 53-op DVE square-accumulate chain doesn't delay the first
            # conv matmul; its one PE matmul slots between chunks 0 and 1) ----
            scal = {}

            def emit_dcoef(dcps):
                # acc[cin,cout] = sum_tap w^2 (DVE), then one matmul with ones
                # reduces over cin: ps_dc[cout,1] = acc.T @ ones.
                ones = small.tile([CIN, 1], dt.float32)
                nc.vector.memset(ones[:], 1.0)
                eps_t = small.tile([COUT, 1], dt.float32)
                nc.vector.memset(eps_t[:], EPS)
                acc = small.tile([CIN, COUT], dt.float32)
                nc.vector.tensor_mul(
                    acc[:], asf32(w_sb[:, 0, :]), asf32(w_sb[:, 0, :])
                )
                for t in range(1, NTAPS):
                    sq = sqp.tile([CIN, COUT], dt.float32)
                    nc.vector.tensor_mul(
                        sq[:], asf32(w_sb[:, t, :]), asf32(w_sb[:, t, :])
                    )
                    nc.vector.tensor_add(acc[:], acc[:], sq[:])
                ps_dc = dcps.tile([COUT, 1], dt.float32)
                nc.tensor.matmul(ps_dc[:], acc[:], ones[:], start=True, stop=True)
                # dscale = sqrt(2) / sqrt(sums + eps)
                rsq = small.tile([COUT, 1], dt.float32)
                nc.scalar.activation(
                    rsq[:], ps_dc[:], mybir.ActivationFunctionType.Sqrt, bias=eps_t[:]
                )
                rec = small.tile([COUT, 1], dt.float32)
                nc.vector.reciprocal(rec[:], rsq[:])
                # epilogue computes u = psum*(sqrt2*dcoef) + sqrt2*bias, then
                # v = max(alpha*u, u) == sqrt2*leaky_relu(psum*dcoef+bias, 0.2)
                dscale = small.tile([COUT, 1], dt.float32)
                nc.scalar.mul(dscale[:], rec[:], S1)
                bias_s = small.tile([COUT, 1], dt.float32)
                nc.scalar.mul(bias_s[:], bias_sb[:], S1)
                scal["dscale"] = dscale
                scal["bias_s"] = bias_s

            # ---- main conv loop (chunk-major: each chunk's 27 matmuls are
            # consecutive, so chunk completions stagger and the epilogues
            # overlap the matmul stream; PE stays at HAM K=8/8 throughout) ----
            with (
                tc.tile_pool(name="ps", bufs=7, space="PSUM") as psp,
                tc.tile_pool(name="dcps", bufs=1, space="PSUM") as dcps,
            ):
                for c in range(NCHUNK):
                    d, h0 = c // 2, (c % 2) * 16
                    ps = psp.tile([COUT, 512], dt.float32, name=f"ps_{c}", tag="ps")
                    valid = [t for t in range(NTAPS) if 0 <= d + t // 9 - 1 < D]
                    for t in valid:
                        kd, kh, kw = t // 9, (t // 3) % 3, t % 3
                        rhs = xpad[:, d + kd - 1, h0 + kh : h0 + kh + 16, kw : kw + 32]
                        nc.tensor.matmul(
                            ps[:],
                            w_sb[:, t, :],
                            rhs,
                            start=(t == valid[0]),
                            stop=(t == valid[-1]),
                        )
                    if c == 0:
                        emit_dcoef(dcps)
                    # epilogue: u = ps*dscale + bias_s; v = max(0.2u, u);
                    # out = clip(v, +-256)
                    u = vp.tile([COUT, 512], dt.float32)
                    nc.vector.tensor_scalar(
                        out=u[:],
                        in0=ps[:],
                        scalar1=scal["dscale"][:],
                        scalar2=scal["bias_s"][:],
                        op0=mybir.AluOpType.mult,
                        op1=mybir.AluOpType.add,
                    )
                    v = vp.tile([COUT, 512], dt.float32, name=f"v_{c}", tag="v")
                    nc.vector.scalar_tensor_tensor(
                        out=v[:],
                        in0=u[:],
                        scalar=ALPHA,
                        in1=u[:],
                        op0=mybir.AluOpType.mult,
                        op1=mybir.AluOpType.max,
                    )
                    oc = op.tile([COUT, 512], dt.float32, name=f"oc_{c}", tag="oc")
                    nc.vector.tensor_scalar(
                        out=oc[:],
                        in0=v[:],
                        scalar1=-CLAMP,
                        scalar2=CLAMP,
                        op0=mybir.AluOpType.max,
                        op1=mybir.AluOpType.min,
                    )
                    nc.sync.dma_start(y_d[:, c, :], oc[:])
    nc.compile()
    return nc


def _get_nc(mode: str):
    if mode not in _CACHED:
        _CACHED[mode] = _build_nc(mode)
    return _CACHED[mode]


def kernel(x: np.ndarray, weight: np.ndarray, bias: np.ndarray) -> np.ndarray:
    global LAST_RESULTS
    mode = MM_MODE
    if mode == "bf16":
        import ml_dtypes

        io = ml_dtypes.bfloat16
    else:
        io = np.float32

    x = np.asarray(x)
    weight = np.asarray(weight, dtype=np.float32)
    bias = np.asarray(bias, dtype=np.float32)

    # [cout, cin, kd, kh, kw] -> [cin, (kd kh kw), cout]
    w_prep = np.ascontiguousarray(
        weight.transpose(1, 2, 3, 4, 0).reshape(CIN, NTAPS, COUT).astype(io)
    )
    b_prep = np.ascontiguousarray(bias.reshape(COUT, 1))

    in_maps = [
        {
            "x": np.ascontiguousarray(x[i].astype(io)),
            "w": w_prep,
            "bias": b_prep,
        }
        for i in range(B)
    ]

    nc = _get_nc(mode)
    trace = bool(int(os.environ.get("CONV_TRACE", "0")))
    res = run_bass_kernel_spmd(
        nc,
        in_maps,
        core_ids=list(range(B)),
        trace=trace,
    )
    LAST_RESULTS = res
    out = np.stack(
        [r["y"].reshape(COUT, D, H, W) for r in res.results], axis=0
    ).astype(np.float32)
    return out
